# revision 29
# baseline (speedup 1.0000x reference)
"""DualGAT (2-hop, 2-graph GAT + gated fuse + MLP) on 8 Trainium2 NeuronCores.

Math used per GAT layer/head (z[v,u] = s_v + t_u):
    exp(LeakyRelu(z, 0.2)) = max(exp(z), exp(0.2 z))        (exact)
    exp(z) = P_v Q_u,  exp(0.2 z) = p_v q_u                 (separable)
    branch select c[v,u] = 1{z > 0}
So with Chat = adjT * c and G = adjT:
    numT @ [Wh|1] = P∘(Chat.T @ (Q∘[Wh|1])) + p∘((G-Chat).T @ (q∘[Wh|1]))

Sharding: v (attention rows) split 8 ways, 384 rows/core; u (neighbors) full.
Feature tensors downstream of attention use padded 4x17 head blocks (17th
lane = softmax denominator); weight rows there are zero-padded.

Schedule highlights:
- compares (tensor_scalar, 4x DVE mode) on DVE; adjacency mask TT chunk-paired
  on DVE with NPOOL chunks per graph on gpsimd, interleaved one per pair so
  neither engine head-of-line blocks; gpsimd chunks' matmuls are emitted last.
- both graphs' epilogues run as ONE batched (FP, 2*VL) pass at layer end.
- hop-1 output is all-gathered in bf16 in 3 column pieces; hop-2's Wh matmuls
  and mask chunks are ordered by piece arrival so they overlap the collective.
- small constants arrive in one packed DMA; adjacency uses the Act HWDGE
  queue so it never delays the weight/feature loads on the SP queue.
"""

import sys
import numpy as np

for _p in ("/opt/trn_rl_repo",):
    if _p not in sys.path:
        sys.path.insert(0, _p)

import ml_dtypes

N = 3072
IN_DIM = 32
HID = 64
HEADS = 4
HD = 16
NCORES = 8
VL = N // NCORES          # 384
P = 128
UC = N // P               # 24
FP = 128                  # padded feature rows: 4 heads x 32
MH = HID // 2
KROWS = [IN_DIM, FP]
BLK = 32
GOFF = [0, 72]
SOFF = [64, 136]
TOFF = [68, 140]
NPOOL = 4                 # chunks per graph whose mask TT runs on gpsimd
NPIECE = 3                # all-gather column pieces
PC = VL // NPIECE         # 128 columns per piece

# packed fp32 constant blob column offsets
_C_WST = 0                              # wst hop-1 pre-image (32, 144)
_C_WT = [[144, 176], [208, 336]]        # WT[l][g] (64, krows)
_C_A = [[464, 472], [480, 488]]         # A[l][g] (64, 8)
_C_QG = [496, 498]                      # qg[l] (128, 2)
_C_MW1 = 500                            # (128, 32)
_C_MB1 = 532                            # (32, 1)
_C_MW2 = 533                            # (32, 1)
_C_MB2 = 534                            # (1, 1)
_C_E17F = 535                           # (4, 128)
_C_ONES = 663                           # (1, 128)
_C_XOWN = 791                           # (32, 384)
_C_TOT = 1175
# packed bf16 blob (128 partitions)
_B_SEL8 = 0                             # (8, 512)
_B_WSTB = 512                           # hop-2 wst pre-image (128, 144)
_B_TOT = 656

DEBUG = False
NO_COLLECTIVE = False

_CACHE = {}


def _build():
    import concourse.bacc as bacc
    import concourse.mybir as mybir
    from concourse.tile import TileContext

    dt = mybir.dt
    op = mybir.AluOpType
    AF = mybir.ActivationFunctionType

    nc = bacc.Bacc("TRN2", target_bir_lowering=False, debug=False,
                   num_devices=NCORES)
    f32r = dt.float32r

    def dram_in(name, shape, dtype=dt.float32):
        return nc.dram_tensor(name, list(shape), dtype, kind="ExternalInput")

    xT_d = dram_in("xT", (IN_DIM, N))
    pk_d = dram_in("pk", (P, _C_TOT))
    pkb_d = dram_in("pkb", (P, _B_TOT), dt.bfloat16)
    adj_d = [dram_in("adjTB_i", (P, UC * VL), dt.bfloat16),
             dram_in("adjTB_c", (P, UC * VL), dt.bfloat16)]
    out_d = nc.dram_tensor("out", [1, VL], dt.float32, kind="ExternalOutput")

    # persistent sbuf
    def sb(name, shape, dtype=dt.float32):
        return nc.alloc_sbuf_tensor(name, list(shape), dtype).ap()

    xT = sb("s_xT", (IN_DIM, N))
    PK = sb("s_pk", (P, _C_TOT))
    PKB = sb("s_pkb", (P, _B_TOT), dt.bfloat16)
    adjTB = [sb(f"s_adjTB{g}", (P, UC * VL), dt.bfloat16) for g in range(2)]
    H1T = sb("s_H1T", (FP, N), dt.bfloat16)
    WH = sb("s_WH", (P, UC * 144))
    QQ = sb("s_QQ", (P, UC * 16))
    WT_u = [[sb(f"s_WTu{g}{h}", (P, UC * 2 * BLK), dt.bfloat16) for h in range(HEADS)] for g in range(2)]
    GW = [sb(f"s_GW{g}", (P, UC * HEADS * BLK), dt.bfloat16) for g in range(2)]
    ST = [sb(f"s_ST{g}", (8, VL), dt.bfloat16) for g in range(2)]
    RRB = sb("s_RRB", (HEADS, 2 * VL))
    CM1B = sb("s_CM1B", (FP, 2 * VL))
    CM3B = sb("s_CM3B", (FP, 2 * VL))
    CPGB = sb("s_CPGB", (FP, 2 * VL))
    HEB = sb("s_HEB", (FP, 2 * VL))
    HF1 = sb("s_HF1", (FP, VL))
    HF2 = sb("s_HF2", (FP, VL))
    AGB = sb("s_AGB", (FP, VL), dt.bfloat16)
    WASB = [[sb(f"s_WA{l}{g}", (KROWS[l], 2 * HEADS)) for g in range(2)] for l in range(2)]

    # views into the packed blobs
    WST = PK[:, _C_WST:_C_WST + 144]
    WSTB = PKB[:, _B_WSTB:_B_WSTB + 144]
    WTSB = [[PK[0:HID, _C_WT[l][g]:_C_WT[l][g] + KROWS[l]] for g in range(2)]
            for l in range(2)]
    ASB = [[PK[0:HID, _C_A[l][g]:_C_A[l][g] + 8] for g in range(2)]
           for l in range(2)]
    QG = [PK[:, _C_QG[l]:_C_QG[l] + 2] for l in range(2)]
    MW1 = PK[:, _C_MW1:_C_MW1 + MH]
    MB1 = PK[0:MH, _C_MB1:_C_MB1 + 1]
    MW2 = PK[0:MH, _C_MW2:_C_MW2 + 1]
    MB2 = PK[0:1, _C_MB2:_C_MB2 + 1]
    E17F = PK[0:HEADS, _C_E17F:_C_E17F + FP]
    ONES68 = PK[0:1, _C_ONES:_C_ONES + FP]
    XOWN = PK[0:IN_DIM, _C_XOWN:_C_XOWN + VL]
    SEL8 = PKB[0:8, _B_SEL8:_B_SEL8 + HEADS * P]

    WH_v = WH.rearrange("p (k c) -> p k c", c=144)
    QQ_v = QQ.rearrange("p (k g j h) -> p k g j h", g=2, j=2, h=HEADS)
    adj_v = [a.rearrange("p (k v) -> p k v", v=VL) for a in adjTB]
    GW_v = [g.rearrange("p (k h c) -> p k h c", h=HEADS, c=BLK) for g in GW]
    WTu_v = [[WT_u[g][h].rearrange("p (k j c) -> p k j c", j=2, c=BLK)
              for h in range(HEADS)] for g in range(2)]

    with TileContext(nc) as tc:
        with tc.tile_pool(name="work", bufs=4) as wp, \
             tc.tile_pool(name="chat", bufs=6) as chp, \
             tc.tile_pool(name="nsb", bufs=4) as nsp, \
             tc.tile_pool(name="small", bufs=4) as smp, \
             tc.tile_pool(name="ps_c", bufs=4, space="PSUM") as ps_c, \
             tc.tile_pool(name="ps_m", bufs=2, space="PSUM") as ps_m, \
             tc.tile_pool(name="dram", bufs=1, space="DRAM") as drp:

            # ---------- loads: 3 DMAs on SP queue, adjacency on Act queue ----
            nc.sync.dma_start(out=PK[:], in_=pk_d.ap())
            nc.sync.dma_start(out=xT[:], in_=xT_d.ap())
            nc.sync.dma_start(out=PKB[:], in_=pkb_d.ap())
            adjspl = UC // 3 * VL
            for piece in range(3):
                sl = slice(piece * adjspl, (piece + 1) * adjspl)
                for g in range(2):
                    nc.sync.dma_start(out=adjTB[g][:, sl],
                                      in_=adj_d[g].ap()[:, sl])
            # zero only the pad columns (16..31 of each 32-block; the den col
            # 16 is rewritten by every build, pads stay zero throughout).
            for g in range(2):
                nc.gpsimd.memset(GW_v[g][:, :, :, 16:32], 0.0)
                for h in range(HEADS):
                    nc.scalar.memzero(WTu_v[g][h][:, :, :, 16:32])

            def prep_weights(l):
                krows = KROWS[l]
                wst = WST if l == 0 else WSTB
                for g in range(2):
                    wa_ps = ps_m.tile([KROWS[1], 2 * HEADS], dt.float32,
                                      tag="m")
                    nc.tensor.matmul(wa_ps[:krows, :],
                                     WTSB[l][g].bitcast(f32r),
                                     ASB[l][g].bitcast(f32r),
                                     start=True, stop=True)
                    nc.scalar.copy(wst[0:krows, SOFF[g]:SOFF[g] + 8],
                                   wa_ps[:krows, :])
                    nc.scalar.copy(WASB[l][g][:], wa_ps[:krows, :])

            def layer(l, HT, hown, hf_out, order):
                """One hop. HT: (krows, N) node-major features (transposed);
                hown: (krows, VL) own-slice fp32 features; hf_out: fused
                output; order: u-chunk processing order."""
                krows = KROWS[l]
                wst = WST if l == 0 else WSTB

                # st+Wh per u-chunk: (krows x 128).T @ (krows x 144)
                for i, k in enumerate(order):
                    stwh = ps_m.tile([P, 144], dt.float32, tag="m")
                    nc.tensor.matmul(stwh[:], HT[:, P * k:P * (k + 1)],
                                     wst[0:krows, :], start=True, stop=True)
                    if i % 3 == 0:
                        nc.scalar.copy(WH_v[:, k, :], stwh[:])
                    elif i % 3 == 1:
                        nc.vector.tensor_copy(out=WH_v[:, k, :], in_=stwh[:])
                    else:
                        nc.gpsimd.tensor_copy(out=WH_v[:, k, :], in_=stwh[:])

                # Q/q
                for g in range(2):
                    tcols = WH_v[:, :, TOFF[g]:TOFF[g] + 4]
                    nc.scalar.activation(QQ_v[:, :, g, 0, :], tcols, AF.Exp)
                    nc.scalar.activation(QQ_v[:, :, g, 1, :], tcols, AF.Exp,
                                         scale=0.2)

                # own-slice s/t rows: ST = WA.T @ hown  (8 x VL)
                for g in range(2):
                    st_ps = ps_m.tile([8, VL], dt.float32, tag="m")
                    nc.tensor.matmul(st_ps[:], WASB[l][g][:].bitcast(f32r),
                                     hown[:].bitcast(f32r),
                                     start=True, stop=True)
                    nc.scalar.copy(ST[g][:], st_ps[:])
                    nc.scalar.activation(RRB[:, g * VL:(g + 1) * VL],
                                         ST[g][0:HEADS, :], AF.Exp, scale=0.8)

                # weight builds: GW (G-stream lhs) on gpsimd; WTu on DVE
                def build_weights(g):
                    nc.gpsimd.tensor_tensor(
                        out=GW_v[g][:, :, :, 0:16],
                        in0=WH_v[:, :, GOFF[g]:GOFF[g] + HID].rearrange(
                            "p k (h d) -> p k h d", d=HD),
                        in1=QQ_v[:, :, g, 1, :][:, :, :, None].to_broadcast(
                            (P, UC, HEADS, HD)),
                        op=op.mult)
                    nc.gpsimd.tensor_copy(out=GW_v[g][:, :, :, 16],
                                          in_=QQ_v[:, :, g, 1, :])
                    for h in range(HEADS):
                        nc.gpsimd.tensor_tensor(
                            out=WTu_v[g][h][:, :, :, 0:16],
                            in0=WH_v[:, :, GOFF[g] + HD * h:
                                     GOFF[g] + HD * h + HD][:, :, None, :]
                                .to_broadcast((P, UC, 2, HD)),
                            in1=QQ_v[:, :, g, :, h][:, :, :, None].to_broadcast(
                                (P, UC, 2, HD)),
                            op=op.mult)
                        nc.gpsimd.tensor_copy(out=WTu_v[g][h][:, :, :, 16],
                                              in_=QQ_v[:, :, g, :, h])

                build_weights(0)

                # mask + matmul streams per graph
                for g in range(2):
                    sbs = []
                    for h in range(HEADS):
                        sb_ps = ps_m.tile([P, VL], dt.float32, tag="m")
                        nc.tensor.matmul(sb_ps[:],
                                         SEL8[:, P * h:P * (h + 1)],
                                         ST[g][:], start=True, stop=True)
                        s_b = nsp.tile([P, VL], dt.bfloat16, tag="ns_b")
                        nc.scalar.copy(s_b[:], sb_ps[:])
                        sbs.append(s_b)

                    psum_cs = []
                    for h in range(HEADS):
                        psum_c = ps_c.tile([2 * BLK, VL], dt.float32,
                                           tag="psum_c")
                        psum_cs.append(psum_c)

                    nmm = [0]

                    def mm_chat(chat, j, k):
                        for h in range(HEADS):
                            nc.tensor.matmul(psum_cs[h][:],
                                             WTu_v[g][h][:, k, :, :],
                                             chat[:, j, h, :],
                                             start=(nmm[0] == 0),
                                             stop=(nmm[0] == UC - 1))
                        nmm[0] += 1

                    pool_chats = []

                    def emit_pool_chunk(k):
                        cb1 = nsp.tile([P, 1, HEADS, VL], dt.bfloat16,
                                       tag=f"cb1{g}", bufs=2, name=f"cb1{g}")
                        for h in range(HEADS):
                            nc.vector.tensor_scalar(
                                cb1[:, 0, h, :], sbs[h][:],
                                WH_v[:, k, TOFF[g] + h:TOFF[g] + h + 1], 0.0,
                                op.add, op.is_gt)
                        chat1 = nsp.tile([P, 1, HEADS, VL], dt.bfloat16,
                                         tag="chat1", bufs=2,
                                         name=f"chat1{g}")
                        nc.gpsimd.tensor_tensor(
                            out=chat1[:], in0=cb1[:],
                            in1=adj_v[g][:, k, :][:, None, None, :]
                                .to_broadcast((P, 1, HEADS, VL)),
                            op=op.mult)
                        pool_chats.append((chat1, k))

                    poolks = order[:NPOOL]
                    dveks = order[NPOOL:]
                    for kp in range(len(dveks) // 2):
                        if kp % 2 == 0 and kp // 2 < NPOOL:
                            emit_pool_chunk(poolks[kp // 2])
                        k0, k1 = dveks[2 * kp], dveks[2 * kp + 1]
                        st = k1 - k0
                        assert st > 0
                        cb2 = chp.tile([P, 2, HEADS, VL], dt.bfloat16,
                                       tag="cb4", bufs=2)
                        for j, k in ((0, k0), (1, k1)):
                            for h in range(HEADS):
                                nc.vector.tensor_scalar(
                                    cb2[:, j, h, :], sbs[h][:],
                                    WH_v[:, k, TOFF[g] + h:TOFF[g] + h + 1],
                                    0.0, op.add, op.is_gt)
                        chat2 = chp.tile([P, 2, HEADS, VL], dt.bfloat16,
                                         tag="chat4", bufs=3)
                        nc.vector.tensor_tensor(
                            out=chat2[:], in0=cb2[:],
                            in1=adj_v[g][:, k0:k1 + 1:st, :][:, :, None, :]
                                .to_broadcast((P, 2, HEADS, VL)),
                            op=op.mult)
                        mm_chat(chat2, 0, k0)
                        mm_chat(chat2, 1, k1)
                    # any leftover pool chunks (when pairs < NPOOL)
                    for k in poolks[len(dveks) // 2:]:
                        emit_pool_chunk(k)

                    # graph 1's weight build fills the tail of graph 0's DVE
                    # stream (it only gates graph 1's matmuls)
                    if g == 0:
                        build_weights(1)

                    # pool-chunk matmuls last
                    for chat1, k in pool_chats:
                        mm_chat(chat1, 0, k)

                    # G-stream: rhs is the resident {0,1} bf16 adjacency
                    psum_g = ps_c.tile([FP, VL], dt.float32, tag="psum_c")
                    for i, k in enumerate(order):
                        nc.tensor.matmul(psum_g[:], GW_v[g][:, k, :, :],
                                         adj_v[g][:, k, :], start=(i == 0),
                                         stop=(i == UC - 1))

                    # ---- per-graph epilogue, entirely on Act/Pool/PE so
                    # graph 0's epilogue overlaps graph 1's DVE mask stream ---
                    gs = slice(g * VL, (g + 1) * VL)
                    for h in range(HEADS):
                        nc.scalar.copy(CM1B[BLK * h:BLK * (h + 1), gs],
                                       psum_cs[h][0:BLK, :])
                        nc.gpsimd.tensor_copy(
                            out=CM3B[BLK * h:BLK * (h + 1), gs],
                            in_=psum_cs[h][BLK:2 * BLK, :])
                    nc.scalar.copy(CPGB[:, gs], psum_g[:])
                    t4 = wp.tile([FP, VL], dt.float32, tag="w")
                    nc.gpsimd.tensor_tensor(out=t4[:], in0=CPGB[:, gs],
                                            in1=CM3B[:, gs], op=op.subtract)
                    rb_ps = ps_m.tile([FP, VL], dt.float32, tag="m")
                    nc.tensor.matmul(rb_ps[:], E17F.bitcast(f32r),
                                     RRB[:, gs].bitcast(f32r),
                                     start=True, stop=True)
                    m1r = wp.tile([FP, VL], dt.float32, tag="w")
                    nc.gpsimd.tensor_tensor(out=m1r[:], in0=CM1B[:, gs],
                                            in1=rb_ps[:], op=op.mult)
                    xx = wp.tile([FP, VL], dt.float32, tag="w")
                    nc.gpsimd.tensor_tensor(out=xx[:], in0=t4[:], in1=m1r[:],
                                            op=op.add)
                    # reciprocal of the 4 denominator lanes via exp(-ln(den))
                    # on Act (keeps DVE free for the other graph's masks)
                    lden = smp.tile([HEADS, VL], dt.float32, tag="s")
                    nc.scalar.activation(lden[:], xx[16::BLK, :], AF.Ln)
                    rda = smp.tile([HEADS, VL], dt.float32, tag="s")
                    nc.scalar.activation(rda[:], lden[:], AF.Exp, scale=-1.0)
                    rd_ps = ps_m.tile([FP, VL], dt.float32, tag="m")
                    nc.tensor.matmul(rd_ps[:], E17F.bitcast(f32r),
                                     rda[:].bitcast(f32r),
                                     start=True, stop=True)
                    hgx = wp.tile([FP, VL], dt.float32, tag="w")
                    nc.gpsimd.tensor_tensor(out=hgx[:], in0=xx[:],
                                            in1=rd_ps[:], op=op.mult)
                    # elu
                    r0 = wp.tile([FP, VL], dt.float32, tag="w")
                    nc.scalar.activation(r0[:], hgx[:], AF.Relu)
                    rn = wp.tile([FP, VL], dt.float32, tag="w")
                    nc.scalar.activation(rn[:], hgx[:], AF.Relu, scale=-1.0)
                    em = wp.tile([FP, VL], dt.float32, tag="w")
                    nc.scalar.activation(em[:], rn[:], AF.Exp, scale=-1.0)
                    nc.gpsimd.scalar_tensor_tensor(
                        out=HEB[:, gs], in0=r0[:], scalar=-1.0, in1=em[:],
                        op0=op.add, op1=op.add)

                # fuse
                ei = []
                for g in range(2):
                    ai_ps = ps_m.tile([1, VL], dt.float32, tag="m")
                    nc.tensor.matmul(ai_ps[:],
                                     QG[l][:, 0 + g:1 + g].bitcast(f32r),
                                     HEB[:, g * VL:(g + 1) * VL].bitcast(f32r),
                                     start=True, stop=True)
                    e = smp.tile([1, VL], dt.float32, tag="s")
                    nc.scalar.activation(e[:], ai_ps[:], AF.Exp)
                    ei.append(e)
                dsum = smp.tile([1, VL], dt.float32, tag="s")
                nc.gpsimd.tensor_tensor(out=dsum[:], in0=ei[0][:],
                                        in1=ei[1][:], op=op.add)
                rdf = smp.tile([1, VL], dt.float32, tag="s")
                nc.vector.reciprocal(rdf[:], dsum[:])
                b0 = smp.tile([1, VL], dt.float32, tag="s")
                nc.gpsimd.tensor_tensor(out=b0[:], in0=ei[0][:], in1=rdf[:],
                                        op=op.mult)
                bib_ps = ps_m.tile([FP, VL], dt.float32, tag="m")
                nc.tensor.matmul(bib_ps[:], ONES68.bitcast(f32r),
                                 b0[:].bitcast(f32r), start=True, stop=True)
                dd = wp.tile([FP, VL], dt.float32, tag="w")
                nc.gpsimd.tensor_tensor(out=dd[:], in0=HEB[:, 0:VL],
                                        in1=HEB[:, VL:2 * VL], op=op.subtract)
                bd = wp.tile([FP, VL], dt.float32, tag="w")
                nc.gpsimd.tensor_tensor(out=bd[:], in0=dd[:], in1=bib_ps[:],
                                        op=op.mult)
                nc.gpsimd.tensor_tensor(out=hf_out[:], in0=HEB[:, VL:2 * VL],
                                        in1=bd[:], op=op.add)

            # ---------------- hop 1 ----------------
            prep_weights(0)
            prep_weights(1)
            layer(0, xT, XOWN, HF1, list(range(UC)))

            # all-gather H1 in bf16, in NPIECE column pieces, each distributed
            # into H1T by a single strided DMA so hop 2 can start per piece.
            nc.scalar.copy(AGB[:], HF1[:])
            for j in range(NPIECE):
                ag_in = drp.tile([FP, PC], dt.bfloat16, name=f"ag_in{j}")
                ag_out = drp.tile([NCORES, FP, PC], dt.bfloat16,
                                  name=f"ag_out{j}")
                nc.sync.dma_start(out=ag_in[:], in_=AGB[:, j * PC:(j + 1) * PC])
                if NO_COLLECTIVE:
                    for c in range(NCORES):
                        eng = nc.sync if c % 2 == 0 else nc.scalar
                        eng.dma_start(
                            out=ag_out.opt().rearrange(
                                "c (f t) -> c f t", t=PC)[c],
                            in_=ag_in[:])
                else:
                    nc.gpsimd.collective_compute(
                        "AllGather", op.bypass,
                        replica_groups=[list(range(NCORES))],
                        ins=[ag_in.opt()], outs=[ag_out.opt()])
                # H1T cols {c*VL + j*PC + t}  <-  ag_out[c, f, t]
                h1_dst = H1T.rearrange("f (c q t) -> f c q t", c=NCORES,
                                       q=NPIECE)[:, :, j, :]
                nc.sync.dma_start(
                    out=h1_dst,
                    in_=ag_out.opt().rearrange("c (f t) -> f c t", t=PC))

            # ---------------- hop 2 (piece-arrival chunk order) ----------
            order2 = [q + NPIECE * c for q in range(NPIECE)
                      for c in range(NCORES)]
            layer(1, H1T, HF1, HF2, order2)

            # ---------------- MLP head ----------------
            h_ps = ps_m.tile([MH, VL], dt.float32, tag="m")
            nc.tensor.matmul(h_ps[:], MW1.bitcast(f32r),
                             HF2[:].bitcast(f32r), start=True, stop=True)
            hd = smp.tile([MH, VL], dt.float32, tag="s")
            nc.scalar.activation(hd[:], h_ps[:], AF.Relu, bias=MB1)
            o_ps = ps_m.tile([1, VL], dt.float32, tag="m")
            nc.tensor.matmul(o_ps[:], MW2.bitcast(f32r),
                             hd[:].bitcast(f32r), start=True, stop=True)
            osb = smp.tile([1, VL], dt.float32, tag="s")
            nc.scalar.activation(osb[:], o_ps[:], AF.Identity, bias=MB2)
            nc.sync.dma_start(out=out_d.ap(), in_=osb[:])

    nc.compile()
    return nc


def _pad_rows(w):
    out = np.zeros((FP,) + w.shape[1:], dtype=np.float32)
    for h in range(HEADS):
        out[BLK * h:BLK * h + 16] = w[16 * h:16 * h + 16]
    return out


def _ahat(a):
    A = np.zeros((HID, 2 * HEADS), dtype=np.float32)
    for h in range(HEADS):
        A[16 * h:16 * h + 16, h] = a[h, :HD]
        A[16 * h:16 * h + 16, HEADS + h] = a[h, HD:]
    return A


def _prep_adj(adj, c):
    """(N,N) int -> per-core (P, UC*VL) bf16 {0,1} chunk layout of adjT."""
    sl = adj[c * VL:(c + 1) * VL, :].T.astype(np.float32)       # (N, VL)
    sl = sl.reshape(UC, P, VL).transpose(1, 0, 2).reshape(P, UC * VL)
    return np.ascontiguousarray(sl).astype(ml_dtypes.bfloat16)


def kernel(**inputs):
    from concourse.bass_utils import run_bass_kernel_spmd

    if "nc" not in _CACHE:
        _CACHE["nc"] = _build()
    nc = _CACHE["nc"]

    f32 = np.float32
    x = np.asarray(inputs["x"], f32)
    adj = [np.asarray(inputs["adj_ind"]), np.asarray(inputs["adj_cor"])]
    W1 = [np.asarray(inputs["W1i"], f32), np.asarray(inputs["W1c"], f32)]
    W2 = [np.asarray(inputs["W2i"], f32), np.asarray(inputs["W2c"], f32)]
    A1 = [np.asarray(inputs["a1i"], f32), np.asarray(inputs["a1c"], f32)]
    A2 = [np.asarray(inputs["a2i"], f32), np.asarray(inputs["a2c"], f32)]
    q1 = [np.asarray(inputs["q1i"], f32), np.asarray(inputs["q1c"], f32)]
    q2 = [np.asarray(inputs["q2i"], f32), np.asarray(inputs["q2c"], f32)]

    # ---- packed fp32 constant blob ----
    pk = np.zeros((P, _C_TOT), dtype=f32)
    for l, (Ws, As) in enumerate(((W1, A1), (W2, A2))):
        kr = KROWS[l]
        for g in range(2):
            W = Ws[g] if l == 0 else _pad_rows(Ws[g])
            if l == 0:
                pk[0:kr, _C_WST + GOFF[g]:_C_WST + GOFF[g] + HID] = W
            pk[0:HID, _C_WT[l][g]:_C_WT[l][g] + kr] = W.T
            pk[0:HID, _C_A[l][g]:_C_A[l][g] + 8] = _ahat(As[g])
    for l, qs in enumerate((q1, q2)):
        pk[:, _C_QG[l]] = _pad_rows(qs[0][:, None])[:, 0]
        pk[:, _C_QG[l] + 1] = _pad_rows(qs[1][:, None])[:, 0]
    pk[:, _C_MW1:_C_MW1 + MH] = _pad_rows(np.asarray(inputs["mlp_w1"], f32))
    pk[0:MH, _C_MB1] = np.asarray(inputs["mlp_b1"], f32)
    pk[0:MH, _C_MW2] = np.asarray(inputs["mlp_w2"], f32)[:, 0]
    pk[0, _C_MB2] = np.asarray(inputs["mlp_b2"], f32).reshape(())
    e17_np = np.zeros((HEADS, FP), dtype=f32)
    for h in range(HEADS):
        e17_np[h, BLK * h:BLK * (h + 1)] = 1.0
    pk[0:HEADS, _C_E17F:_C_E17F + FP] = e17_np
    pk[0, _C_ONES:_C_ONES + FP] = 1.0

    # ---- packed bf16 blob: sel8 + hop-2 wst pre-image ----
    pkb = np.zeros((P, _B_TOT), dtype=np.float32)
    for h in range(HEADS):
        pkb[h, _B_SEL8 + P * h:_B_SEL8 + P * (h + 1)] = 1.0
    for g in range(2):
        pkb[:, _B_WSTB + GOFF[g]:_B_WSTB + GOFF[g] + HID] = _pad_rows(W2[g])
    pkb = pkb.astype(ml_dtypes.bfloat16)

    common = {"xT": np.ascontiguousarray(x.T), "pkb": pkb}

    in_maps = []
    for c in range(NCORES):
        m = dict(common)
        pkc = pk.copy()
        pkc[0:IN_DIM, _C_XOWN:_C_XOWN + VL] = x[c * VL:(c + 1) * VL, :].T
        m["pk"] = pkc
        m["adjTB_i"] = _prep_adj(adj[0], c)
        m["adjTB_c"] = _prep_adj(adj[1], c)
        in_maps.append(m)

    res = run_bass_kernel_spmd(nc, in_maps, core_ids=list(range(NCORES)))
    out = np.concatenate([r["out"][0] for r in res.results])[:, None]
    return out.astype(np.float32)


if __name__ == "__main__":
    _CACHE["nc"] = _build()
    print("build ok")


# revision 30
# speedup vs baseline: 1.2515x; 1.2515x over previous
"""DualGAT (2-hop, 2-graph GAT + gated fuse + MLP) on 8 Trainium2 NeuronCores.

Math used per GAT layer/head (z[v,u] = s_v + t_u):
    exp(LeakyRelu(z, 0.2)) = max(exp(z), exp(0.2 z))        (exact)
    exp(z) = P_v Q_u,  exp(0.2 z) = p_v q_u                 (separable)
    branch select c[v,u] = 1{z > 0}
So with Chat = adjT * c and G = adjT:
    numT @ [Wh|1] = P∘(Chat.T @ (Q∘[Wh|1])) + p∘((G-Chat).T @ (q∘[Wh|1]))

Sharding: v (attention rows) split 8 ways, 384 rows/core; u (neighbors) full.
Feature tensors downstream of attention use padded 4x17 head blocks (17th
lane = softmax denominator); weight rows there are zero-padded.

Schedule highlights:
- compares (tensor_scalar, 4x DVE mode) on DVE; adjacency mask TT chunk-paired
  on DVE with NPOOL chunks per graph on gpsimd, interleaved one per pair so
  neither engine head-of-line blocks; gpsimd chunks' matmuls are emitted last.
- both graphs' epilogues run as ONE batched (FP, 2*VL) pass at layer end.
- hop-1 output is all-gathered in bf16 in 3 column pieces; hop-2's Wh matmuls
  and mask chunks are ordered by piece arrival so they overlap the collective.
- small constants arrive in one packed DMA; adjacency uses the Act HWDGE
  queue so it never delays the weight/feature loads on the SP queue.
"""

import sys
import numpy as np

for _p in ("/opt/trn_rl_repo",):
    if _p not in sys.path:
        sys.path.insert(0, _p)

import ml_dtypes

N = 3072
IN_DIM = 32
HID = 64
HEADS = 4
HD = 16
NCORES = 8
VL = N // NCORES          # 384
P = 128
UC = N // P               # 24
FP = 128                  # padded feature rows: 4 heads x 32
MH = HID // 2
KROWS = [IN_DIM, FP]
BLK = 32
GOFF = [0, 72]
SOFF = [64, 136]
TOFF = [68, 140]
NPOOL = 0                 # chunks per graph whose mask TT runs on gpsimd
NPIECE = 3                # all-gather column pieces
PC = VL // NPIECE         # 128 columns per piece

# packed fp32 constant blob column offsets
_C_WST = 0                              # wst hop-1 pre-image (32, 144)
_C_WT = [[144, 176], [208, 336]]        # WT[l][g] (64, krows)
_C_A = [[464, 472], [480, 488]]         # A[l][g] (64, 8)
_C_QG = [496, 498]                      # qg[l] (128, 2)
_C_MW1 = 500                            # (128, 32)
_C_MB1 = 532                            # (32, 1)
_C_MW2 = 533                            # (32, 1)
_C_MB2 = 534                            # (1, 1)
_C_E17F = 535                           # (4, 128)
_C_ONES = 663                           # (1, 128)
_C_XOWN = 791                           # (32, 384)
_C_TOT = 1175
# packed bf16 blob (128 partitions)
_B_SEL8 = 0                             # (8, 512)
_B_WSTB = 512                           # hop-2 wst pre-image (128, 144)
_B_TOT = 656

DEBUG = False
NO_COLLECTIVE = False

_CACHE = {}


def _build():
    import concourse.bacc as bacc
    import concourse.mybir as mybir
    from concourse.tile import TileContext

    dt = mybir.dt
    op = mybir.AluOpType
    AF = mybir.ActivationFunctionType

    nc = bacc.Bacc("TRN2", target_bir_lowering=False, debug=False,
                   num_devices=NCORES)
    f32r = dt.float32r

    def dram_in(name, shape, dtype=dt.float32):
        return nc.dram_tensor(name, list(shape), dtype, kind="ExternalInput")

    xT_d = dram_in("xT", (IN_DIM, N))
    pk_d = dram_in("pk", (P, _C_TOT))
    pkb_d = dram_in("pkb", (P, _B_TOT), dt.bfloat16)
    adj_d = [dram_in("adjTB_i", (P, UC * VL), dt.bfloat16),
             dram_in("adjTB_c", (P, UC * VL), dt.bfloat16)]
    out_d = nc.dram_tensor("out", [1, VL], dt.float32, kind="ExternalOutput")

    # persistent sbuf
    def sb(name, shape, dtype=dt.float32):
        return nc.alloc_sbuf_tensor(name, list(shape), dtype).ap()

    xT = sb("s_xT", (IN_DIM, N))
    PK = sb("s_pk", (P, _C_TOT))
    PKB = sb("s_pkb", (P, _B_TOT), dt.bfloat16)
    adjTB = [sb(f"s_adjTB{g}", (P, UC * VL), dt.bfloat16) for g in range(2)]
    H1T = sb("s_H1T", (FP, N), dt.bfloat16)
    WH = sb("s_WH", (P, UC * 144))
    QQ = sb("s_QQ", (P, UC * 16))
    WT_u = [[sb(f"s_WTu{g}{h}", (P, UC * 2 * BLK), dt.bfloat16) for h in range(HEADS)] for g in range(2)]
    GW = [sb(f"s_GW{g}", (P, UC * HEADS * BLK), dt.bfloat16) for g in range(2)]
    ST = [sb(f"s_ST{g}", (8, VL), dt.bfloat16) for g in range(2)]
    RRB = sb("s_RRB", (HEADS, 2 * VL))
    CM1B = sb("s_CM1B", (FP, 2 * VL))
    CM3B = sb("s_CM3B", (FP, 2 * VL))
    CPGB = sb("s_CPGB", (FP, 2 * VL))
    HEB = sb("s_HEB", (FP, 2 * VL))
    HF1 = sb("s_HF1", (FP, VL))
    HF2 = sb("s_HF2", (FP, VL))
    AGB = sb("s_AGB", (FP, VL), dt.bfloat16)
    WASB = [[sb(f"s_WA{l}{g}", (KROWS[l], 2 * HEADS)) for g in range(2)] for l in range(2)]

    # views into the packed blobs
    WST = PK[:, _C_WST:_C_WST + 144]
    WSTB = PKB[:, _B_WSTB:_B_WSTB + 144]
    WTSB = [[PK[0:HID, _C_WT[l][g]:_C_WT[l][g] + KROWS[l]] for g in range(2)]
            for l in range(2)]
    ASB = [[PK[0:HID, _C_A[l][g]:_C_A[l][g] + 8] for g in range(2)]
           for l in range(2)]
    QG = [PK[:, _C_QG[l]:_C_QG[l] + 2] for l in range(2)]
    MW1 = PK[:, _C_MW1:_C_MW1 + MH]
    MB1 = PK[0:MH, _C_MB1:_C_MB1 + 1]
    MW2 = PK[0:MH, _C_MW2:_C_MW2 + 1]
    MB2 = PK[0:1, _C_MB2:_C_MB2 + 1]
    E17F = PK[0:HEADS, _C_E17F:_C_E17F + FP]
    ONES68 = PK[0:1, _C_ONES:_C_ONES + FP]
    XOWN = PK[0:IN_DIM, _C_XOWN:_C_XOWN + VL]
    SEL8 = PKB[0:8, _B_SEL8:_B_SEL8 + HEADS * P]

    WH_v = WH.rearrange("p (k c) -> p k c", c=144)
    QQ_v = QQ.rearrange("p (k g j h) -> p k g j h", g=2, j=2, h=HEADS)
    adj_v = [a.rearrange("p (k v) -> p k v", v=VL) for a in adjTB]
    GW_v = [g.rearrange("p (k h c) -> p k h c", h=HEADS, c=BLK) for g in GW]
    WTu_v = [[WT_u[g][h].rearrange("p (k j c) -> p k j c", j=2, c=BLK)
              for h in range(HEADS)] for g in range(2)]

    with TileContext(nc) as tc:
        with tc.tile_pool(name="work", bufs=4) as wp, \
             tc.tile_pool(name="chat", bufs=6) as chp, \
             tc.tile_pool(name="nsb", bufs=4) as nsp, \
             tc.tile_pool(name="small", bufs=4) as smp, \
             tc.tile_pool(name="ps_c", bufs=4, space="PSUM") as ps_c, \
             tc.tile_pool(name="ps_m", bufs=2, space="PSUM") as ps_m, \
             tc.tile_pool(name="dram", bufs=1, space="DRAM") as drp:

            # ---------- loads: 3 DMAs on SP queue, adjacency on Act queue ----
            nc.sync.dma_start(out=PK[:], in_=pk_d.ap())
            nc.sync.dma_start(out=xT[:], in_=xT_d.ap())
            nc.sync.dma_start(out=PKB[:], in_=pkb_d.ap())
            adjspl = UC // 3 * VL
            for piece in range(3):
                sl = slice(piece * adjspl, (piece + 1) * adjspl)
                for g in range(2):
                    nc.sync.dma_start(out=adjTB[g][:, sl],
                                      in_=adj_d[g].ap()[:, sl])
            # zero only the pad columns (16..31 of each 32-block; the den col
            # 16 is rewritten by every build, pads stay zero throughout).
            for g in range(2):
                nc.gpsimd.memset(GW_v[g][:, :, :, 16:32], 0.0)
                for h in range(HEADS):
                    nc.scalar.memzero(WTu_v[g][h][:, :, :, 16:32])

            def prep_weights(l):
                krows = KROWS[l]
                wst = WST if l == 0 else WSTB
                for g in range(2):
                    wa_ps = ps_m.tile([KROWS[1], 2 * HEADS], dt.float32,
                                      tag="m")
                    nc.tensor.matmul(wa_ps[:krows, :],
                                     WTSB[l][g].bitcast(f32r),
                                     ASB[l][g].bitcast(f32r),
                                     start=True, stop=True)
                    nc.scalar.copy(wst[0:krows, SOFF[g]:SOFF[g] + 8],
                                   wa_ps[:krows, :])
                    nc.scalar.copy(WASB[l][g][:], wa_ps[:krows, :])

            def layer(l, HT, hown, hf_out, order):
                """One hop. HT: (krows, N) node-major features (transposed);
                hown: (krows, VL) own-slice fp32 features; hf_out: fused
                output; order: u-chunk processing order."""
                krows = KROWS[l]
                wst = WST if l == 0 else WSTB

                # st+Wh per u-chunk: (krows x 128).T @ (krows x 144)
                for i, k in enumerate(order):
                    stwh = ps_m.tile([P, 144], dt.float32, tag="m")
                    nc.tensor.matmul(stwh[:], HT[:, P * k:P * (k + 1)],
                                     wst[0:krows, :], start=True, stop=True)
                    if i % 3 == 0:
                        nc.scalar.copy(WH_v[:, k, :], stwh[:])
                    elif i % 3 == 1:
                        nc.vector.tensor_copy(out=WH_v[:, k, :], in_=stwh[:])
                    else:
                        nc.gpsimd.tensor_copy(out=WH_v[:, k, :], in_=stwh[:])

                # Q/q
                for g in range(2):
                    tcols = WH_v[:, :, TOFF[g]:TOFF[g] + 4]
                    nc.scalar.activation(QQ_v[:, :, g, 0, :], tcols, AF.Exp)
                    nc.scalar.activation(QQ_v[:, :, g, 1, :], tcols, AF.Exp,
                                         scale=0.2)

                # own-slice s/t rows: ST = WA.T @ hown  (8 x VL)
                for g in range(2):
                    st_ps = ps_m.tile([8, VL], dt.float32, tag="m")
                    nc.tensor.matmul(st_ps[:], WASB[l][g][:].bitcast(f32r),
                                     hown[:].bitcast(f32r),
                                     start=True, stop=True)
                    nc.scalar.copy(ST[g][:], st_ps[:])
                    nc.scalar.activation(RRB[:, g * VL:(g + 1) * VL],
                                         ST[g][0:HEADS, :], AF.Exp, scale=0.8)

                # weight builds: GW (G-stream lhs) on gpsimd; WTu on DVE
                def build_weights(g):
                    nc.gpsimd.tensor_tensor(
                        out=GW_v[g][:, :, :, 0:16],
                        in0=WH_v[:, :, GOFF[g]:GOFF[g] + HID].rearrange(
                            "p k (h d) -> p k h d", d=HD),
                        in1=QQ_v[:, :, g, 1, :][:, :, :, None].to_broadcast(
                            (P, UC, HEADS, HD)),
                        op=op.mult)
                    nc.gpsimd.tensor_copy(out=GW_v[g][:, :, :, 16],
                                          in_=QQ_v[:, :, g, 1, :])
                    for h in range(HEADS):
                        nc.gpsimd.tensor_tensor(
                            out=WTu_v[g][h][:, :, :, 0:16],
                            in0=WH_v[:, :, GOFF[g] + HD * h:
                                     GOFF[g] + HD * h + HD][:, :, None, :]
                                .to_broadcast((P, UC, 2, HD)),
                            in1=QQ_v[:, :, g, :, h][:, :, :, None].to_broadcast(
                                (P, UC, 2, HD)),
                            op=op.mult)
                        nc.gpsimd.tensor_copy(out=WTu_v[g][h][:, :, :, 16],
                                              in_=QQ_v[:, :, g, :, h])

                build_weights(0)

                # mask + matmul streams per graph
                for g in range(2):
                    sbs = []
                    for h in range(HEADS):
                        sb_ps = ps_m.tile([P, VL], dt.float32, tag="m")
                        nc.tensor.matmul(sb_ps[:],
                                         SEL8[:, P * h:P * (h + 1)],
                                         ST[g][:], start=True, stop=True)
                        s_b = nsp.tile([P, VL], dt.bfloat16, tag="ns_b")
                        nc.scalar.copy(s_b[:], sb_ps[:])
                        sbs.append(s_b)

                    psum_cs = []
                    for h in range(HEADS):
                        psum_c = ps_c.tile([2 * BLK, VL], dt.float32,
                                           tag="psum_c")
                        psum_cs.append(psum_c)

                    nmm = [0]

                    def mm_chat(chat, j, k):
                        for h in range(HEADS):
                            nc.tensor.matmul(psum_cs[h][:],
                                             WTu_v[g][h][:, k, :, :],
                                             chat[:, j, h, :],
                                             start=(nmm[0] == 0),
                                             stop=(nmm[0] == UC - 1))
                        nmm[0] += 1

                    pool_chats = []

                    def emit_pool_chunk(k):
                        cb1 = nsp.tile([P, 1, HEADS, VL], dt.bfloat16,
                                       tag=f"cb1{g}", bufs=2, name=f"cb1{g}")
                        for h in range(HEADS):
                            nc.vector.tensor_scalar(
                                cb1[:, 0, h, :], sbs[h][:],
                                WH_v[:, k, TOFF[g] + h:TOFF[g] + h + 1], 0.0,
                                op.add, op.is_gt)
                        chat1 = nsp.tile([P, 1, HEADS, VL], dt.bfloat16,
                                         tag="chat1", bufs=2,
                                         name=f"chat1{g}")
                        nc.gpsimd.tensor_tensor(
                            out=chat1[:], in0=cb1[:],
                            in1=adj_v[g][:, k, :][:, None, None, :]
                                .to_broadcast((P, 1, HEADS, VL)),
                            op=op.mult)
                        pool_chats.append((chat1, k))

                    poolks = order[:NPOOL]
                    dveks = order[NPOOL:]
                    for kp in range(len(dveks) // 2):
                        if kp < NPOOL:
                            emit_pool_chunk(poolks[kp])
                        k0, k1 = dveks[2 * kp], dveks[2 * kp + 1]
                        st = k1 - k0
                        assert st > 0
                        cb2 = chp.tile([P, 2, HEADS, VL], dt.bfloat16,
                                       tag="cb4", bufs=2)
                        for j, k in ((0, k0), (1, k1)):
                            for h in range(HEADS):
                                nc.vector.tensor_scalar(
                                    cb2[:, j, h, :], sbs[h][:],
                                    WH_v[:, k, TOFF[g] + h:TOFF[g] + h + 1],
                                    0.0, op.add, op.is_gt)
                        chat2 = chp.tile([P, 2, HEADS, VL], dt.bfloat16,
                                         tag="chat4", bufs=3)
                        nc.vector.tensor_tensor(
                            out=chat2[:], in0=cb2[:],
                            in1=adj_v[g][:, k0:k1 + 1:st, :][:, :, None, :]
                                .to_broadcast((P, 2, HEADS, VL)),
                            op=op.mult)
                        mm_chat(chat2, 0, k0)
                        mm_chat(chat2, 1, k1)
                    # any leftover pool chunks (when pairs < NPOOL)
                    for k in poolks[len(dveks) // 2:]:
                        emit_pool_chunk(k)

                    # graph 1's weight build fills the tail of graph 0's DVE
                    # stream (it only gates graph 1's matmuls)
                    if g == 0:
                        build_weights(1)

                    # pool-chunk matmuls last
                    for chat1, k in pool_chats:
                        mm_chat(chat1, 0, k)

                    # G-stream: rhs is the resident {0,1} bf16 adjacency
                    psum_g = ps_c.tile([FP, VL], dt.float32, tag="psum_c")
                    for i, k in enumerate(order):
                        nc.tensor.matmul(psum_g[:], GW_v[g][:, k, :, :],
                                         adj_v[g][:, k, :], start=(i == 0),
                                         stop=(i == UC - 1))

                    # bank results into the both-graph epilogue tensors
                    gs = slice(g * VL, (g + 1) * VL)
                    for h in range(HEADS):
                        nc.scalar.copy(CM1B[BLK * h:BLK * (h + 1), gs],
                                       psum_cs[h][0:BLK, :])
                        nc.gpsimd.tensor_copy(
                            out=CM3B[BLK * h:BLK * (h + 1), gs],
                            in_=psum_cs[h][BLK:2 * BLK, :])
                    nc.scalar.copy(CPGB[:, gs], psum_g[:])

                # ---- merged epilogue over both graphs (FP, 2*VL) ----
                W2C = 2 * VL
                t4 = wp.tile([FP, W2C], dt.float32, tag="w")
                nc.vector.tensor_tensor(out=t4[:], in0=CPGB[:], in1=CM3B[:],
                                        op=op.subtract)
                rb_ps = ps_m.tile([FP, W2C], dt.float32, tag="mb", bufs=1)
                nc.tensor.matmul(rb_ps[:], E17F.bitcast(f32r),
                                 RRB[:].bitcast(f32r), start=True, stop=True)
                m1r = wp.tile([FP, W2C], dt.float32, tag="w")
                nc.vector.tensor_tensor(out=m1r[:], in0=CM1B[:], in1=rb_ps[:],
                                        op=op.mult)
                xx = wp.tile([FP, W2C], dt.float32, tag="w")
                nc.vector.tensor_tensor(out=xx[:], in0=t4[:], in1=m1r[:],
                                        op=op.add)
                rda = smp.tile([HEADS, W2C], dt.float32, tag="s")
                nc.vector.reciprocal(rda[:], xx[16::BLK, :])
                rd_ps = ps_m.tile([FP, W2C], dt.float32, tag="mb", bufs=1)
                nc.tensor.matmul(rd_ps[:], E17F.bitcast(f32r),
                                 rda[:].bitcast(f32r), start=True, stop=True)
                hgx = wp.tile([FP, W2C], dt.float32, tag="w")
                nc.vector.tensor_tensor(out=hgx[:], in0=xx[:], in1=rd_ps[:],
                                        op=op.mult)

                # elu
                r0 = wp.tile([FP, W2C], dt.float32, tag="w")
                nc.scalar.activation(r0[:], hgx[:], AF.Relu)
                rn = wp.tile([FP, W2C], dt.float32, tag="w")
                nc.scalar.activation(rn[:], hgx[:], AF.Relu, scale=-1.0)
                em = wp.tile([FP, W2C], dt.float32, tag="w")
                nc.scalar.activation(em[:], rn[:], AF.Exp, scale=-1.0)
                nc.vector.scalar_tensor_tensor(
                    out=HEB[:], in0=r0[:], scalar=-1.0, in1=em[:],
                    op0=op.add, op1=op.add)

                # fuse
                ei = []
                for g in range(2):
                    ai_ps = ps_m.tile([1, VL], dt.float32, tag="m")
                    nc.tensor.matmul(ai_ps[:],
                                     QG[l][:, 0 + g:1 + g].bitcast(f32r),
                                     HEB[:, g * VL:(g + 1) * VL].bitcast(f32r),
                                     start=True, stop=True)
                    e = smp.tile([1, VL], dt.float32, tag="s")
                    nc.scalar.activation(e[:], ai_ps[:], AF.Exp)
                    ei.append(e)
                dsum = smp.tile([1, VL], dt.float32, tag="s")
                nc.vector.tensor_tensor(out=dsum[:], in0=ei[0][:], in1=ei[1][:],
                                        op=op.add)
                rdf = smp.tile([1, VL], dt.float32, tag="s")
                nc.vector.reciprocal(rdf[:], dsum[:])
                b0 = smp.tile([1, VL], dt.float32, tag="s")
                nc.vector.tensor_tensor(out=b0[:], in0=ei[0][:], in1=rdf[:],
                                        op=op.mult)
                bib_ps = ps_m.tile([FP, VL], dt.float32, tag="m")
                nc.tensor.matmul(bib_ps[:], ONES68.bitcast(f32r),
                                 b0[:].bitcast(f32r), start=True, stop=True)
                dd = wp.tile([FP, VL], dt.float32, tag="w")
                nc.vector.tensor_tensor(out=dd[:], in0=HEB[:, 0:VL],
                                        in1=HEB[:, VL:2 * VL], op=op.subtract)
                bd = wp.tile([FP, VL], dt.float32, tag="w")
                nc.vector.tensor_tensor(out=bd[:], in0=dd[:], in1=bib_ps[:],
                                        op=op.mult)
                nc.vector.tensor_tensor(out=hf_out[:], in0=HEB[:, VL:2 * VL],
                                        in1=bd[:], op=op.add)

            # ---------------- hop 1 ----------------
            prep_weights(0)
            prep_weights(1)
            layer(0, xT, XOWN, HF1, list(range(UC)))

            # all-gather H1 in bf16, in NPIECE column pieces, each distributed
            # into H1T by a single strided DMA so hop 2 can start per piece.
            nc.scalar.copy(AGB[:], HF1[:])
            for j in range(NPIECE):
                ag_in = drp.tile([FP, PC], dt.bfloat16, name=f"ag_in{j}")
                ag_out = drp.tile([NCORES, FP, PC], dt.bfloat16,
                                  name=f"ag_out{j}")
                nc.sync.dma_start(out=ag_in[:], in_=AGB[:, j * PC:(j + 1) * PC])
                if NO_COLLECTIVE:
                    for c in range(NCORES):
                        eng = nc.sync if c % 2 == 0 else nc.scalar
                        eng.dma_start(
                            out=ag_out.opt().rearrange(
                                "c (f t) -> c f t", t=PC)[c],
                            in_=ag_in[:])
                else:
                    nc.gpsimd.collective_compute(
                        "AllGather", op.bypass,
                        replica_groups=[list(range(NCORES))],
                        ins=[ag_in.opt()], outs=[ag_out.opt()])
                # H1T cols {c*VL + j*PC + t}  <-  ag_out[c, f, t]
                h1_dst = H1T.rearrange("f (c q t) -> f c q t", c=NCORES,
                                       q=NPIECE)[:, :, j, :]
                nc.sync.dma_start(
                    out=h1_dst,
                    in_=ag_out.opt().rearrange("c (f t) -> f c t", t=PC))

            # ---------------- hop 2 (piece-arrival chunk order) ----------
            order2 = [q + NPIECE * c for q in range(NPIECE)
                      for c in range(NCORES)]
            layer(1, H1T, HF1, HF2, order2)

            # ---------------- MLP head ----------------
            h_ps = ps_m.tile([MH, VL], dt.float32, tag="m")
            nc.tensor.matmul(h_ps[:], MW1.bitcast(f32r),
                             HF2[:].bitcast(f32r), start=True, stop=True)
            hd = smp.tile([MH, VL], dt.float32, tag="s")
            nc.scalar.activation(hd[:], h_ps[:], AF.Relu, bias=MB1)
            o_ps = ps_m.tile([1, VL], dt.float32, tag="m")
            nc.tensor.matmul(o_ps[:], MW2.bitcast(f32r),
                             hd[:].bitcast(f32r), start=True, stop=True)
            osb = smp.tile([1, VL], dt.float32, tag="s")
            nc.scalar.activation(osb[:], o_ps[:], AF.Identity, bias=MB2)
            nc.sync.dma_start(out=out_d.ap(), in_=osb[:])

    nc.compile()
    return nc


def _pad_rows(w):
    out = np.zeros((FP,) + w.shape[1:], dtype=np.float32)
    for h in range(HEADS):
        out[BLK * h:BLK * h + 16] = w[16 * h:16 * h + 16]
    return out


def _ahat(a):
    A = np.zeros((HID, 2 * HEADS), dtype=np.float32)
    for h in range(HEADS):
        A[16 * h:16 * h + 16, h] = a[h, :HD]
        A[16 * h:16 * h + 16, HEADS + h] = a[h, HD:]
    return A


def _prep_adj(adj, c):
    """(N,N) int -> per-core (P, UC*VL) bf16 {0,1} chunk layout of adjT."""
    sl = adj[c * VL:(c + 1) * VL, :].T.astype(np.float32)       # (N, VL)
    sl = sl.reshape(UC, P, VL).transpose(1, 0, 2).reshape(P, UC * VL)
    return np.ascontiguousarray(sl).astype(ml_dtypes.bfloat16)


def kernel(**inputs):
    from concourse.bass_utils import run_bass_kernel_spmd

    if "nc" not in _CACHE:
        _CACHE["nc"] = _build()
    nc = _CACHE["nc"]

    f32 = np.float32
    x = np.asarray(inputs["x"], f32)
    adj = [np.asarray(inputs["adj_ind"]), np.asarray(inputs["adj_cor"])]
    W1 = [np.asarray(inputs["W1i"], f32), np.asarray(inputs["W1c"], f32)]
    W2 = [np.asarray(inputs["W2i"], f32), np.asarray(inputs["W2c"], f32)]
    A1 = [np.asarray(inputs["a1i"], f32), np.asarray(inputs["a1c"], f32)]
    A2 = [np.asarray(inputs["a2i"], f32), np.asarray(inputs["a2c"], f32)]
    q1 = [np.asarray(inputs["q1i"], f32), np.asarray(inputs["q1c"], f32)]
    q2 = [np.asarray(inputs["q2i"], f32), np.asarray(inputs["q2c"], f32)]

    # ---- packed fp32 constant blob ----
    pk = np.zeros((P, _C_TOT), dtype=f32)
    for l, (Ws, As) in enumerate(((W1, A1), (W2, A2))):
        kr = KROWS[l]
        for g in range(2):
            W = Ws[g] if l == 0 else _pad_rows(Ws[g])
            if l == 0:
                pk[0:kr, _C_WST + GOFF[g]:_C_WST + GOFF[g] + HID] = W
            pk[0:HID, _C_WT[l][g]:_C_WT[l][g] + kr] = W.T
            pk[0:HID, _C_A[l][g]:_C_A[l][g] + 8] = _ahat(As[g])
    for l, qs in enumerate((q1, q2)):
        pk[:, _C_QG[l]] = _pad_rows(qs[0][:, None])[:, 0]
        pk[:, _C_QG[l] + 1] = _pad_rows(qs[1][:, None])[:, 0]
    pk[:, _C_MW1:_C_MW1 + MH] = _pad_rows(np.asarray(inputs["mlp_w1"], f32))
    pk[0:MH, _C_MB1] = np.asarray(inputs["mlp_b1"], f32)
    pk[0:MH, _C_MW2] = np.asarray(inputs["mlp_w2"], f32)[:, 0]
    pk[0, _C_MB2] = np.asarray(inputs["mlp_b2"], f32).reshape(())
    e17_np = np.zeros((HEADS, FP), dtype=f32)
    for h in range(HEADS):
        e17_np[h, BLK * h:BLK * (h + 1)] = 1.0
    pk[0:HEADS, _C_E17F:_C_E17F + FP] = e17_np
    pk[0, _C_ONES:_C_ONES + FP] = 1.0

    # ---- packed bf16 blob: sel8 + hop-2 wst pre-image ----
    pkb = np.zeros((P, _B_TOT), dtype=np.float32)
    for h in range(HEADS):
        pkb[h, _B_SEL8 + P * h:_B_SEL8 + P * (h + 1)] = 1.0
    for g in range(2):
        pkb[:, _B_WSTB + GOFF[g]:_B_WSTB + GOFF[g] + HID] = _pad_rows(W2[g])
    pkb = pkb.astype(ml_dtypes.bfloat16)

    common = {"xT": np.ascontiguousarray(x.T), "pkb": pkb}

    in_maps = []
    for c in range(NCORES):
        m = dict(common)
        pkc = pk.copy()
        pkc[0:IN_DIM, _C_XOWN:_C_XOWN + VL] = x[c * VL:(c + 1) * VL, :].T
        m["pk"] = pkc
        m["adjTB_i"] = _prep_adj(adj[0], c)
        m["adjTB_c"] = _prep_adj(adj[1], c)
        in_maps.append(m)

    res = run_bass_kernel_spmd(nc, in_maps, core_ids=list(range(NCORES)))
    out = np.concatenate([r["out"][0] for r in res.results])[:, None]
    return out.astype(np.float32)


if __name__ == "__main__":
    _CACHE["nc"] = _build()
    print("build ok")


# revision 31
# speedup vs baseline: 1.2532x; 1.0013x over previous
"""DualGAT (2-hop, 2-graph GAT + gated fuse + MLP) on 8 Trainium2 NeuronCores.

Math used per GAT layer/head (z[v,u] = s_v + t_u):
    exp(LeakyRelu(z, 0.2)) = max(exp(z), exp(0.2 z))        (exact)
    exp(z) = P_v Q_u,  exp(0.2 z) = p_v q_u                 (separable)
    branch select c[v,u] = 1{z > 0}
So with Chat = adjT * c and G = adjT:
    numT @ [Wh|1] = P∘(Chat.T @ (Q∘[Wh|1])) + p∘((G-Chat).T @ (q∘[Wh|1]))

Sharding: v (attention rows) split 8 ways, 384 rows/core; u (neighbors) full.
Feature tensors downstream of attention use padded 4x17 head blocks (17th
lane = softmax denominator); weight rows there are zero-padded.

Schedule highlights:
- compares (tensor_scalar, 4x DVE mode) on DVE; adjacency mask TT chunk-paired
  on DVE with NPOOL chunks per graph on gpsimd, interleaved one per pair so
  neither engine head-of-line blocks; gpsimd chunks' matmuls are emitted last.
- both graphs' epilogues run as ONE batched (FP, 2*VL) pass at layer end.
- hop-1 output is all-gathered in bf16 in 3 column pieces; hop-2's Wh matmuls
  and mask chunks are ordered by piece arrival so they overlap the collective.
- small constants arrive in one packed DMA; adjacency uses the Act HWDGE
  queue so it never delays the weight/feature loads on the SP queue.
"""

import sys
import numpy as np

for _p in ("/opt/trn_rl_repo",):
    if _p not in sys.path:
        sys.path.insert(0, _p)

import ml_dtypes

N = 3072
IN_DIM = 32
HID = 64
HEADS = 4
HD = 16
NCORES = 8
VL = N // NCORES          # 384
P = 128
UC = N // P               # 24
FP = 128                  # padded feature rows: 4 heads x 32
MH = HID // 2
KROWS = [IN_DIM, FP]
BLK = 32
GOFF = [0, 72]
SOFF = [64, 136]
TOFF = [68, 140]
NPOOL = 0                 # chunks per graph whose mask TT runs on gpsimd
NPIECE = 3                # all-gather column pieces
PC = VL // NPIECE         # 128 columns per piece

# packed fp32 constant blob column offsets
_C_WST = 0                              # wst hop-1 pre-image (32, 144)
_C_WT = [[144, 176], [208, 336]]        # WT[l][g] (64, krows)
_C_A = [[464, 472], [480, 488]]         # A[l][g] (64, 8)
_C_QG = [496, 498]                      # qg[l] (128, 2)
_C_MW1 = 500                            # (128, 32)
_C_MB1 = 532                            # (32, 1)
_C_MW2 = 533                            # (32, 1)
_C_MB2 = 534                            # (1, 1)
_C_E17F = 535                           # (4, 128)
_C_ONES = 663                           # (1, 128)
_C_XOWN = 791                           # (32, 384)
_C_TOT = 1175
# packed bf16 blob (128 partitions)
_B_SEL8 = 0                             # (8, 512)
_B_WSTB = 512                           # hop-2 wst pre-image (128, 144)
_B_TOT = 656

DEBUG = False
NO_COLLECTIVE = False

_CACHE = {}


def _build():
    import concourse.bacc as bacc
    import concourse.mybir as mybir
    from concourse.tile import TileContext

    dt = mybir.dt
    op = mybir.AluOpType
    AF = mybir.ActivationFunctionType

    nc = bacc.Bacc("TRN2", target_bir_lowering=False, debug=False,
                   num_devices=NCORES)
    f32r = dt.float32r

    def dram_in(name, shape, dtype=dt.float32):
        return nc.dram_tensor(name, list(shape), dtype, kind="ExternalInput")

    xT_d = dram_in("xT", (IN_DIM, N))
    pk_d = dram_in("pk", (P, _C_TOT))
    pkb_d = dram_in("pkb", (P, _B_TOT), dt.bfloat16)
    adj_d = [dram_in("adjTB_i", (P, UC * VL), dt.bfloat16),
             dram_in("adjTB_c", (P, UC * VL), dt.bfloat16)]
    out_d = nc.dram_tensor("out", [1, VL], dt.float32, kind="ExternalOutput")

    # persistent sbuf
    def sb(name, shape, dtype=dt.float32):
        return nc.alloc_sbuf_tensor(name, list(shape), dtype).ap()

    xT = sb("s_xT", (IN_DIM, N))
    PK = sb("s_pk", (P, _C_TOT))
    PKB = sb("s_pkb", (P, _B_TOT), dt.bfloat16)
    adjTB = [sb(f"s_adjTB{g}", (P, UC * VL), dt.bfloat16) for g in range(2)]
    H1T = sb("s_H1T", (FP, N), dt.bfloat16)
    WH = sb("s_WH", (P, UC * 144))
    QQ = sb("s_QQ", (P, UC * 16))
    WT_u = [[sb(f"s_WTu{g}{h}", (P, UC * 2 * BLK), dt.bfloat16) for h in range(HEADS)] for g in range(2)]
    GW = [sb(f"s_GW{g}", (P, UC * HEADS * BLK), dt.bfloat16) for g in range(2)]
    ST = [sb(f"s_ST{g}", (8, VL), dt.bfloat16) for g in range(2)]
    RRB = sb("s_RRB", (HEADS, 2 * VL))
    CM1B = sb("s_CM1B", (FP, 2 * VL))
    CM3B = sb("s_CM3B", (FP, 2 * VL))
    CPGB = sb("s_CPGB", (FP, 2 * VL))
    HEB = sb("s_HEB", (FP, 2 * VL))
    HF1 = sb("s_HF1", (FP, VL))
    HF2 = sb("s_HF2", (FP, VL))
    AGB = sb("s_AGB", (FP, VL), dt.bfloat16)
    WASB = [[sb(f"s_WA{l}{g}", (KROWS[l], 2 * HEADS)) for g in range(2)] for l in range(2)]

    # views into the packed blobs
    WST = PK[:, _C_WST:_C_WST + 144]
    WSTB = PKB[:, _B_WSTB:_B_WSTB + 144]
    WTSB = [[PK[0:HID, _C_WT[l][g]:_C_WT[l][g] + KROWS[l]] for g in range(2)]
            for l in range(2)]
    ASB = [[PK[0:HID, _C_A[l][g]:_C_A[l][g] + 8] for g in range(2)]
           for l in range(2)]
    QG = [PK[:, _C_QG[l]:_C_QG[l] + 2] for l in range(2)]
    MW1 = PK[:, _C_MW1:_C_MW1 + MH]
    MB1 = PK[0:MH, _C_MB1:_C_MB1 + 1]
    MW2 = PK[0:MH, _C_MW2:_C_MW2 + 1]
    MB2 = PK[0:1, _C_MB2:_C_MB2 + 1]
    E17F = PK[0:HEADS, _C_E17F:_C_E17F + FP]
    ONES68 = PK[0:1, _C_ONES:_C_ONES + FP]
    XOWN = PK[0:IN_DIM, _C_XOWN:_C_XOWN + VL]
    SEL8 = PKB[0:8, _B_SEL8:_B_SEL8 + HEADS * P]

    WH_v = WH.rearrange("p (k c) -> p k c", c=144)
    QQ_v = QQ.rearrange("p (k g j h) -> p k g j h", g=2, j=2, h=HEADS)
    adj_v = [a.rearrange("p (k v) -> p k v", v=VL) for a in adjTB]
    GW_v = [g.rearrange("p (k h c) -> p k h c", h=HEADS, c=BLK) for g in GW]
    WTu_v = [[WT_u[g][h].rearrange("p (k j c) -> p k j c", j=2, c=BLK)
              for h in range(HEADS)] for g in range(2)]

    with TileContext(nc) as tc:
        with tc.tile_pool(name="work", bufs=4) as wp, \
             tc.tile_pool(name="chat", bufs=6) as chp, \
             tc.tile_pool(name="nsb", bufs=4) as nsp, \
             tc.tile_pool(name="small", bufs=4) as smp, \
             tc.tile_pool(name="ps_c", bufs=4, space="PSUM") as ps_c, \
             tc.tile_pool(name="ps_m", bufs=2, space="PSUM") as ps_m, \
             tc.tile_pool(name="dram", bufs=1, space="DRAM") as drp:

            # ---------- loads: 3 DMAs on SP queue, adjacency on Act queue ----
            nc.sync.dma_start(out=PK[:], in_=pk_d.ap())
            nc.sync.dma_start(out=xT[:], in_=xT_d.ap())
            nc.sync.dma_start(out=PKB[:], in_=pkb_d.ap())
            adjspl = UC // 3 * VL
            for piece in range(3):
                sl = slice(piece * adjspl, (piece + 1) * adjspl)
                for g in range(2):
                    nc.sync.dma_start(out=adjTB[g][:, sl],
                                      in_=adj_d[g].ap()[:, sl])
            # zero only the pad columns (16..31 of each 32-block; the den col
            # 16 is rewritten by every build, pads stay zero throughout).
            for g in range(2):
                nc.gpsimd.memset(GW_v[g][:, :, :, 16:32], 0.0)
                for h in range(HEADS):
                    nc.scalar.memzero(WTu_v[g][h][:, :, :, 16:32])

            def prep_weights(l):
                krows = KROWS[l]
                wst = WST if l == 0 else WSTB
                for g in range(2):
                    wa_ps = ps_m.tile([KROWS[1], 2 * HEADS], dt.float32,
                                      tag="m")
                    nc.tensor.matmul(wa_ps[:krows, :],
                                     WTSB[l][g].bitcast(f32r),
                                     ASB[l][g].bitcast(f32r),
                                     start=True, stop=True)
                    nc.scalar.copy(wst[0:krows, SOFF[g]:SOFF[g] + 8],
                                   wa_ps[:krows, :])
                    nc.scalar.copy(WASB[l][g][:], wa_ps[:krows, :])

            def layer(l, HT, hown, hf_out, order):
                """One hop. HT: (krows, N) node-major features (transposed);
                hown: (krows, VL) own-slice fp32 features; hf_out: fused
                output; order: u-chunk processing order."""
                krows = KROWS[l]
                wst = WST if l == 0 else WSTB

                # st+Wh per u-chunk: (krows x 128).T @ (krows x 144)
                for i, k in enumerate(order):
                    stwh = ps_m.tile([P, 144], dt.float32, tag="m")
                    nc.tensor.matmul(stwh[:], HT[:, P * k:P * (k + 1)],
                                     wst[0:krows, :], start=True, stop=True)
                    if i % 3 == 0:
                        nc.scalar.copy(WH_v[:, k, :], stwh[:])
                    elif i % 3 == 1:
                        nc.vector.tensor_copy(out=WH_v[:, k, :], in_=stwh[:])
                    else:
                        nc.gpsimd.tensor_copy(out=WH_v[:, k, :], in_=stwh[:])

                # Q/q
                for g in range(2):
                    tcols = WH_v[:, :, TOFF[g]:TOFF[g] + 4]
                    nc.scalar.activation(QQ_v[:, :, g, 0, :], tcols, AF.Exp)
                    nc.scalar.activation(QQ_v[:, :, g, 1, :], tcols, AF.Exp,
                                         scale=0.2)

                # own-slice s/t rows: ST = WA.T @ hown  (8 x VL)
                for g in range(2):
                    st_ps = ps_m.tile([8, VL], dt.float32, tag="m")
                    nc.tensor.matmul(st_ps[:], WASB[l][g][:].bitcast(f32r),
                                     hown[:].bitcast(f32r),
                                     start=True, stop=True)
                    nc.scalar.copy(ST[g][:], st_ps[:])
                    nc.scalar.activation(RRB[:, g * VL:(g + 1) * VL],
                                         ST[g][0:HEADS, :], AF.Exp, scale=0.8)

                # weight builds: GW (G-stream lhs) on gpsimd; WTu on DVE
                def build_weights(g):
                    nc.gpsimd.tensor_tensor(
                        out=GW_v[g][:, :, :, 0:16],
                        in0=WH_v[:, :, GOFF[g]:GOFF[g] + HID].rearrange(
                            "p k (h d) -> p k h d", d=HD),
                        in1=QQ_v[:, :, g, 1, :][:, :, :, None].to_broadcast(
                            (P, UC, HEADS, HD)),
                        op=op.mult)
                    nc.gpsimd.tensor_copy(out=GW_v[g][:, :, :, 16],
                                          in_=QQ_v[:, :, g, 1, :])
                    for h in range(HEADS):
                        nc.gpsimd.tensor_tensor(
                            out=WTu_v[g][h][:, :, :, 0:16],
                            in0=WH_v[:, :, GOFF[g] + HD * h:
                                     GOFF[g] + HD * h + HD][:, :, None, :]
                                .to_broadcast((P, UC, 2, HD)),
                            in1=QQ_v[:, :, g, :, h][:, :, :, None].to_broadcast(
                                (P, UC, 2, HD)),
                            op=op.mult)
                        nc.gpsimd.tensor_copy(out=WTu_v[g][h][:, :, :, 16],
                                              in_=QQ_v[:, :, g, :, h])

                build_weights(0)

                # broadcast s rows for BOTH graphs up front so nothing in
                # a graph's epilogue can head-of-line block the next stream
                sbs_g = []
                for g in range(2):
                    row = []
                    for h in range(HEADS):
                        sb_ps = ps_m.tile([P, VL], dt.float32, tag="m")
                        nc.tensor.matmul(sb_ps[:],
                                         SEL8[:, P * h:P * (h + 1)],
                                         ST[g][:], start=True, stop=True)
                        s_b = nsp.tile([P, VL], dt.bfloat16, tag="ns_b",
                                       bufs=8)
                        nc.scalar.copy(s_b[:], sb_ps[:])
                        row.append(s_b)
                    sbs_g.append(row)

                # mask + matmul streams per graph
                for g in range(2):
                    sbs = sbs_g[g]

                    psum_cs = []
                    for h in range(HEADS):
                        psum_c = ps_c.tile([2 * BLK, VL], dt.float32,
                                           tag="psum_c")
                        psum_cs.append(psum_c)

                    nmm = [0]

                    def mm_chat(chat, j, k):
                        for h in range(HEADS):
                            nc.tensor.matmul(psum_cs[h][:],
                                             WTu_v[g][h][:, k, :, :],
                                             chat[:, j, h, :],
                                             start=(nmm[0] == 0),
                                             stop=(nmm[0] == UC - 1))
                        nmm[0] += 1

                    pool_chats = []

                    def emit_pool_chunk(k):
                        cb1 = nsp.tile([P, 1, HEADS, VL], dt.bfloat16,
                                       tag=f"cb1{g}", bufs=2, name=f"cb1{g}")
                        for h in range(HEADS):
                            nc.vector.tensor_scalar(
                                cb1[:, 0, h, :], sbs[h][:],
                                WH_v[:, k, TOFF[g] + h:TOFF[g] + h + 1], 0.0,
                                op.add, op.is_gt)
                        chat1 = nsp.tile([P, 1, HEADS, VL], dt.bfloat16,
                                         tag="chat1", bufs=2,
                                         name=f"chat1{g}")
                        nc.gpsimd.tensor_tensor(
                            out=chat1[:], in0=cb1[:],
                            in1=adj_v[g][:, k, :][:, None, None, :]
                                .to_broadcast((P, 1, HEADS, VL)),
                            op=op.mult)
                        pool_chats.append((chat1, k))

                    poolks = order[:NPOOL]
                    dveks = order[NPOOL:]
                    for kp in range(len(dveks) // 2):
                        if kp < NPOOL:
                            emit_pool_chunk(poolks[kp])
                        k0, k1 = dveks[2 * kp], dveks[2 * kp + 1]
                        st = k1 - k0
                        assert st > 0
                        cb2 = chp.tile([P, 2, HEADS, VL], dt.bfloat16,
                                       tag="cb4", bufs=2)
                        for j, k in ((0, k0), (1, k1)):
                            for h in range(HEADS):
                                nc.vector.tensor_scalar(
                                    cb2[:, j, h, :], sbs[h][:],
                                    WH_v[:, k, TOFF[g] + h:TOFF[g] + h + 1],
                                    0.0, op.add, op.is_gt)
                        chat2 = chp.tile([P, 2, HEADS, VL], dt.bfloat16,
                                         tag="chat4", bufs=3)
                        nc.vector.tensor_tensor(
                            out=chat2[:], in0=cb2[:],
                            in1=adj_v[g][:, k0:k1 + 1:st, :][:, :, None, :]
                                .to_broadcast((P, 2, HEADS, VL)),
                            op=op.mult)
                        mm_chat(chat2, 0, k0)
                        mm_chat(chat2, 1, k1)
                    # any leftover pool chunks (when pairs < NPOOL)
                    for k in poolks[len(dveks) // 2:]:
                        emit_pool_chunk(k)

                    # graph 1's weight build fills the tail of graph 0's DVE
                    # stream (it only gates graph 1's matmuls)
                    if g == 0:
                        build_weights(1)

                    # pool-chunk matmuls last
                    for chat1, k in pool_chats:
                        mm_chat(chat1, 0, k)

                    # G-stream: rhs is the resident {0,1} bf16 adjacency
                    psum_g = ps_c.tile([FP, VL], dt.float32, tag="psum_c")
                    for i, k in enumerate(order):
                        nc.tensor.matmul(psum_g[:], GW_v[g][:, k, :, :],
                                         adj_v[g][:, k, :], start=(i == 0),
                                         stop=(i == UC - 1))

                    # bank results into the both-graph epilogue tensors
                    gs = slice(g * VL, (g + 1) * VL)
                    for h in range(HEADS):
                        nc.scalar.copy(CM1B[BLK * h:BLK * (h + 1), gs],
                                       psum_cs[h][0:BLK, :])
                        nc.gpsimd.tensor_copy(
                            out=CM3B[BLK * h:BLK * (h + 1), gs],
                            in_=psum_cs[h][BLK:2 * BLK, :])
                    nc.scalar.copy(CPGB[:, gs], psum_g[:])

                # ---- merged epilogue over both graphs (FP, 2*VL) ----
                W2C = 2 * VL
                t4 = wp.tile([FP, W2C], dt.float32, tag="w")
                nc.vector.tensor_tensor(out=t4[:], in0=CPGB[:], in1=CM3B[:],
                                        op=op.subtract)
                rb_ps = ps_m.tile([FP, W2C], dt.float32, tag="mb", bufs=1)
                nc.tensor.matmul(rb_ps[:], E17F.bitcast(f32r),
                                 RRB[:].bitcast(f32r), start=True, stop=True)
                m1r = wp.tile([FP, W2C], dt.float32, tag="w")
                nc.vector.tensor_tensor(out=m1r[:], in0=CM1B[:], in1=rb_ps[:],
                                        op=op.mult)
                xx = wp.tile([FP, W2C], dt.float32, tag="w")
                nc.vector.tensor_tensor(out=xx[:], in0=t4[:], in1=m1r[:],
                                        op=op.add)
                rda = smp.tile([HEADS, W2C], dt.float32, tag="s")
                nc.vector.reciprocal(rda[:], xx[16::BLK, :])
                rd_ps = ps_m.tile([FP, W2C], dt.float32, tag="mb", bufs=1)
                nc.tensor.matmul(rd_ps[:], E17F.bitcast(f32r),
                                 rda[:].bitcast(f32r), start=True, stop=True)
                hgx = wp.tile([FP, W2C], dt.float32, tag="w")
                nc.vector.tensor_tensor(out=hgx[:], in0=xx[:], in1=rd_ps[:],
                                        op=op.mult)

                # elu
                r0 = wp.tile([FP, W2C], dt.float32, tag="w")
                nc.scalar.activation(r0[:], hgx[:], AF.Relu)
                rn = wp.tile([FP, W2C], dt.float32, tag="w")
                nc.scalar.activation(rn[:], hgx[:], AF.Relu, scale=-1.0)
                em = wp.tile([FP, W2C], dt.float32, tag="w")
                nc.scalar.activation(em[:], rn[:], AF.Exp, scale=-1.0)
                nc.vector.scalar_tensor_tensor(
                    out=HEB[:], in0=r0[:], scalar=-1.0, in1=em[:],
                    op0=op.add, op1=op.add)

                # fuse
                ei = []
                for g in range(2):
                    ai_ps = ps_m.tile([1, VL], dt.float32, tag="m")
                    nc.tensor.matmul(ai_ps[:],
                                     QG[l][:, 0 + g:1 + g].bitcast(f32r),
                                     HEB[:, g * VL:(g + 1) * VL].bitcast(f32r),
                                     start=True, stop=True)
                    e = smp.tile([1, VL], dt.float32, tag="s")
                    nc.scalar.activation(e[:], ai_ps[:], AF.Exp)
                    ei.append(e)
                dsum = smp.tile([1, VL], dt.float32, tag="s")
                nc.vector.tensor_tensor(out=dsum[:], in0=ei[0][:], in1=ei[1][:],
                                        op=op.add)
                rdf = smp.tile([1, VL], dt.float32, tag="s")
                nc.vector.reciprocal(rdf[:], dsum[:])
                b0 = smp.tile([1, VL], dt.float32, tag="s")
                nc.vector.tensor_tensor(out=b0[:], in0=ei[0][:], in1=rdf[:],
                                        op=op.mult)
                bib_ps = ps_m.tile([FP, VL], dt.float32, tag="m")
                nc.tensor.matmul(bib_ps[:], ONES68.bitcast(f32r),
                                 b0[:].bitcast(f32r), start=True, stop=True)
                dd = wp.tile([FP, VL], dt.float32, tag="w")
                nc.vector.tensor_tensor(out=dd[:], in0=HEB[:, 0:VL],
                                        in1=HEB[:, VL:2 * VL], op=op.subtract)
                bd = wp.tile([FP, VL], dt.float32, tag="w")
                nc.vector.tensor_tensor(out=bd[:], in0=dd[:], in1=bib_ps[:],
                                        op=op.mult)
                nc.vector.tensor_tensor(out=hf_out[:], in0=HEB[:, VL:2 * VL],
                                        in1=bd[:], op=op.add)

            # ---------------- hop 1 ----------------
            prep_weights(0)
            prep_weights(1)
            layer(0, xT, XOWN, HF1, list(range(UC)))

            # all-gather H1 in bf16, in NPIECE column pieces, each distributed
            # into H1T by a single strided DMA so hop 2 can start per piece.
            nc.scalar.copy(AGB[:], HF1[:])
            for j in range(NPIECE):
                ag_in = drp.tile([FP, PC], dt.bfloat16, name=f"ag_in{j}")
                ag_out = drp.tile([NCORES, FP, PC], dt.bfloat16,
                                  name=f"ag_out{j}")
                nc.sync.dma_start(out=ag_in[:], in_=AGB[:, j * PC:(j + 1) * PC])
                if NO_COLLECTIVE:
                    for c in range(NCORES):
                        eng = nc.sync if c % 2 == 0 else nc.scalar
                        eng.dma_start(
                            out=ag_out.opt().rearrange(
                                "c (f t) -> c f t", t=PC)[c],
                            in_=ag_in[:])
                else:
                    nc.gpsimd.collective_compute(
                        "AllGather", op.bypass,
                        replica_groups=[list(range(NCORES))],
                        ins=[ag_in.opt()], outs=[ag_out.opt()])
                # H1T cols {c*VL + j*PC + t}  <-  ag_out[c, f, t]
                h1_dst = H1T.rearrange("f (c q t) -> f c q t", c=NCORES,
                                       q=NPIECE)[:, :, j, :]
                nc.sync.dma_start(
                    out=h1_dst,
                    in_=ag_out.opt().rearrange("c (f t) -> f c t", t=PC))

            # ---------------- hop 2 (piece-arrival chunk order) ----------
            order2 = [q + NPIECE * c for q in range(NPIECE)
                      for c in range(NCORES)]
            layer(1, H1T, HF1, HF2, order2)

            # ---------------- MLP head ----------------
            h_ps = ps_m.tile([MH, VL], dt.float32, tag="m")
            nc.tensor.matmul(h_ps[:], MW1.bitcast(f32r),
                             HF2[:].bitcast(f32r), start=True, stop=True)
            hd = smp.tile([MH, VL], dt.float32, tag="s")
            nc.scalar.activation(hd[:], h_ps[:], AF.Relu, bias=MB1)
            o_ps = ps_m.tile([1, VL], dt.float32, tag="m")
            nc.tensor.matmul(o_ps[:], MW2.bitcast(f32r),
                             hd[:].bitcast(f32r), start=True, stop=True)
            osb = smp.tile([1, VL], dt.float32, tag="s")
            nc.scalar.activation(osb[:], o_ps[:], AF.Identity, bias=MB2)
            nc.sync.dma_start(out=out_d.ap(), in_=osb[:])

    nc.compile()
    return nc


def _pad_rows(w):
    out = np.zeros((FP,) + w.shape[1:], dtype=np.float32)
    for h in range(HEADS):
        out[BLK * h:BLK * h + 16] = w[16 * h:16 * h + 16]
    return out


def _ahat(a):
    A = np.zeros((HID, 2 * HEADS), dtype=np.float32)
    for h in range(HEADS):
        A[16 * h:16 * h + 16, h] = a[h, :HD]
        A[16 * h:16 * h + 16, HEADS + h] = a[h, HD:]
    return A


def _prep_adj(adj, c):
    """(N,N) int -> per-core (P, UC*VL) bf16 {0,1} chunk layout of adjT."""
    sl = adj[c * VL:(c + 1) * VL, :].T.astype(np.float32)       # (N, VL)
    sl = sl.reshape(UC, P, VL).transpose(1, 0, 2).reshape(P, UC * VL)
    return np.ascontiguousarray(sl).astype(ml_dtypes.bfloat16)


def kernel(**inputs):
    from concourse.bass_utils import run_bass_kernel_spmd

    if "nc" not in _CACHE:
        _CACHE["nc"] = _build()
    nc = _CACHE["nc"]

    f32 = np.float32
    x = np.asarray(inputs["x"], f32)
    adj = [np.asarray(inputs["adj_ind"]), np.asarray(inputs["adj_cor"])]
    W1 = [np.asarray(inputs["W1i"], f32), np.asarray(inputs["W1c"], f32)]
    W2 = [np.asarray(inputs["W2i"], f32), np.asarray(inputs["W2c"], f32)]
    A1 = [np.asarray(inputs["a1i"], f32), np.asarray(inputs["a1c"], f32)]
    A2 = [np.asarray(inputs["a2i"], f32), np.asarray(inputs["a2c"], f32)]
    q1 = [np.asarray(inputs["q1i"], f32), np.asarray(inputs["q1c"], f32)]
    q2 = [np.asarray(inputs["q2i"], f32), np.asarray(inputs["q2c"], f32)]

    # ---- packed fp32 constant blob ----
    pk = np.zeros((P, _C_TOT), dtype=f32)
    for l, (Ws, As) in enumerate(((W1, A1), (W2, A2))):
        kr = KROWS[l]
        for g in range(2):
            W = Ws[g] if l == 0 else _pad_rows(Ws[g])
            if l == 0:
                pk[0:kr, _C_WST + GOFF[g]:_C_WST + GOFF[g] + HID] = W
            pk[0:HID, _C_WT[l][g]:_C_WT[l][g] + kr] = W.T
            pk[0:HID, _C_A[l][g]:_C_A[l][g] + 8] = _ahat(As[g])
    for l, qs in enumerate((q1, q2)):
        pk[:, _C_QG[l]] = _pad_rows(qs[0][:, None])[:, 0]
        pk[:, _C_QG[l] + 1] = _pad_rows(qs[1][:, None])[:, 0]
    pk[:, _C_MW1:_C_MW1 + MH] = _pad_rows(np.asarray(inputs["mlp_w1"], f32))
    pk[0:MH, _C_MB1] = np.asarray(inputs["mlp_b1"], f32)
    pk[0:MH, _C_MW2] = np.asarray(inputs["mlp_w2"], f32)[:, 0]
    pk[0, _C_MB2] = np.asarray(inputs["mlp_b2"], f32).reshape(())
    e17_np = np.zeros((HEADS, FP), dtype=f32)
    for h in range(HEADS):
        e17_np[h, BLK * h:BLK * (h + 1)] = 1.0
    pk[0:HEADS, _C_E17F:_C_E17F + FP] = e17_np
    pk[0, _C_ONES:_C_ONES + FP] = 1.0

    # ---- packed bf16 blob: sel8 + hop-2 wst pre-image ----
    pkb = np.zeros((P, _B_TOT), dtype=np.float32)
    for h in range(HEADS):
        pkb[h, _B_SEL8 + P * h:_B_SEL8 + P * (h + 1)] = 1.0
    for g in range(2):
        pkb[:, _B_WSTB + GOFF[g]:_B_WSTB + GOFF[g] + HID] = _pad_rows(W2[g])
    pkb = pkb.astype(ml_dtypes.bfloat16)

    common = {"xT": np.ascontiguousarray(x.T), "pkb": pkb}

    in_maps = []
    for c in range(NCORES):
        m = dict(common)
        pkc = pk.copy()
        pkc[0:IN_DIM, _C_XOWN:_C_XOWN + VL] = x[c * VL:(c + 1) * VL, :].T
        m["pk"] = pkc
        m["adjTB_i"] = _prep_adj(adj[0], c)
        m["adjTB_c"] = _prep_adj(adj[1], c)
        in_maps.append(m)

    res = run_bass_kernel_spmd(nc, in_maps, core_ids=list(range(NCORES)))
    out = np.concatenate([r["out"][0] for r in res.results])[:, None]
    return out.astype(np.float32)


if __name__ == "__main__":
    _CACHE["nc"] = _build()
    print("build ok")
